# revision 9
# baseline (speedup 1.0000x reference)
"""Trainium2 Bass kernel for Mesh_Reduced.knn_interpolate (k=3 inverse-distance
interpolation from 2048 pivotal nodes onto 65536 mesh nodes).

Strategy (per sharding hint): shard query nodes (pos_y / output rows) across
the 8 NeuronCores; replicate the small pivotal set (x, pos_x) on every core.

Per-core pipeline, for each chunk of 128 queries:
  1. PE matmul computes a score matrix s[q, n] = 2*y.x - ||x||^2  (= ||y||^2 -
     d2): the larger the score, the nearer the source.  K=4 augmented matmul
     ([y0,y1,y2,1] x [2x0,2x1,2x2,-||x||^2]) adds the per-source bias for free.
  2. ScalarE copies PSUM -> SBUF.
  3. VectorE Max8 gives the top-8 scores per query (descending); MaxIndex
     gives their source indices.  k=3 <= 8, so one pass each suffices; the
     hardware tie semantics (distinct indices for duplicated values) match
     jax.lax.top_k.
  4. GPSIMD indirect DMA gathers the 3 selected source feature rows per query
     from DRAM (batched over 8 chunks: one descriptor per row).
  5. VectorE computes d2_j = ||y||^2 - v_j, w_j = 1/clip(d2_j, 1e-16), and the
     weighted feature average (all ops batched over 8 chunks).
"""

import numpy as np

import concourse.bacc as bacc
import concourse.bass as bass
import concourse.mybir as mybir
import concourse.tile as tile
from concourse.bass import IndirectOffsetOnAxis
from concourse.bass_utils import run_bass_kernel_spmd

N_CORES = 8
NX = 2048          # pivotal (source) nodes
NY = 65536         # mesh (query) nodes
C = 16             # feature channels
K = 3
P = 128            # SBUF partitions (queries per chunk)
NY_SHARD = NY // N_CORES          # 8192 queries per core
N_CHUNKS = NY_SHARD // P          # 64 chunks per core
BATCH = 8                         # chunks handled per batched epilogue
N_BATCHES = N_CHUNKS // BATCH
MM_N = 512                        # moving-operand cols per matmul (1 PSUM bank)
KDIM = 21                         # compensated-bf16 contraction rows

f32 = mybir.dt.float32
bf16 = mybir.dt.bfloat16
u32 = mybir.dt.uint32

_BUILT = None  # cached (nc) so repeat kernel() calls don't re-trace


def _build_kernel():
    nc = bacc.Bacc("TRN2", target_bir_lowering=False, debug=False)

    yt_d = nc.dram_tensor("yt", [KDIM, NY_SHARD], bf16, kind="ExternalInput")
    xt_d = nc.dram_tensor("xt", [KDIM, NX], bf16, kind="ExternalInput")
    ysq_d = nc.dram_tensor("ysq", [P, N_CHUNKS], f32, kind="ExternalInput")
    xf_d = nc.dram_tensor("xf", [NX, C], f32, kind="ExternalInput")
    out_d = nc.dram_tensor("out", [NY_SHARD, C], f32, kind="ExternalOutput")

    with tile.TileContext(nc) as tc:
        with (
            tc.tile_pool(name="const", bufs=1) as const,
            tc.tile_pool(name="psum", bufs=2, space="PSUM") as psum,
            tc.tile_pool(name="sbig", bufs=3) as sbig,
            tc.tile_pool(name="small", bufs=2) as small,
        ):
            yt_sb = const.tile([KDIM, NY_SHARD], bf16)
            nc.sync.dma_start(yt_sb[:], yt_d[:])
            xt_sb = const.tile([KDIM, NX], bf16)
            nc.sync.dma_start(xt_sb[:], xt_d[:])
            ysq_sb = const.tile([P, N_CHUNKS], f32)
            nc.sync.dma_start(ysq_sb[:], ysq_d[:])

            # out viewed so partition = query-within-chunk: row = c*P + p
            out_v = out_d[:].rearrange("(c p) f -> p c f", p=P)

            for b in range(N_BATCHES):
                vb = small.tile([P, BATCH * 8], f32, tag="vb")
                ib = small.tile([P, BATCH * 8], u32, tag="ib")
                xg = small.tile([P, BATCH, K, C], f32, tag="xg")
                for cc in range(BATCH):
                    c = b * BATCH + cc
                    ps = psum.tile([P, NX], f32, tag="ps")
                    for i in range(NX // MM_N):
                        nc.tensor.matmul(
                            ps[:, i * MM_N:(i + 1) * MM_N],
                            lhsT=yt_sb[:, c * P:(c + 1) * P],
                            rhs=xt_sb[:, i * MM_N:(i + 1) * MM_N],
                            start=True,
                            stop=True,
                        )
                    s_sb = sbig.tile([P, NX], f32, tag="s")
                    nc.scalar.copy(out=s_sb[:], in_=ps[:])
                    nc.vector.max(out=vb[:, cc * 8:(cc + 1) * 8], in_=s_sb[:])
                    nc.vector.max_index(
                        out=ib[:, cc * 8:(cc + 1) * 8],
                        in_max=vb[:, cc * 8:(cc + 1) * 8],
                        in_values=s_sb[:],
                    )
                    # HW indirect DMA: exactly one offset per partition per op
                    for j in range(K):
                        nc.gpsimd.indirect_dma_start(
                            out=xg[:, cc, j, :],
                            out_offset=None,
                            in_=xf_d[:],
                            in_offset=IndirectOffsetOnAxis(
                                ap=ib[:, cc * 8 + j:cc * 8 + j + 1], axis=0
                            ),
                        )

                # ---- batched epilogue over BATCH chunks ----
                v3 = vb[:].rearrange("p (b e) -> p b e", e=8)[:, :, 0:K]

                # d2_j = ||y||^2 - v_j  (clipped), w_j = 1/d2_j
                d2 = small.tile([P, BATCH, K], f32, tag="d2")
                ysq_bc = (
                    ysq_sb[:, b * BATCH:(b + 1) * BATCH]
                    .unsqueeze(-1)
                    .to_broadcast([P, BATCH, K])
                )
                nc.vector.tensor_tensor(
                    out=d2[:], in0=ysq_bc, in1=v3, op=mybir.AluOpType.subtract
                )
                nc.vector.tensor_scalar_max(out=d2[:], in0=d2[:], scalar1=1e-16)
                w = small.tile([P, BATCH, K], f32, tag="w")
                nc.vector.reciprocal(out=w[:], in_=d2[:])

                prod = small.tile([P, BATCH, K, C], f32, tag="prod")
                nc.vector.tensor_tensor(
                    out=prod[:],
                    in0=xg[:],
                    in1=w[:].unsqueeze(-1).to_broadcast([P, BATCH, K, C]),
                    op=mybir.AluOpType.mult,
                )
                num = small.tile([P, BATCH, C], f32, tag="num")
                nc.vector.tensor_reduce(
                    out=num[:],
                    in_=prod[:].transpose([0, 1, 3, 2]),
                    axis=mybir.AxisListType.X,
                    op=mybir.AluOpType.add,
                )
                den = small.tile([P, BATCH], f32, tag="den")
                nc.vector.tensor_reduce(
                    out=den[:], in_=w[:], axis=mybir.AxisListType.X,
                    op=mybir.AluOpType.add,
                )
                invd = small.tile([P, BATCH], f32, tag="invd")
                nc.vector.reciprocal(out=invd[:], in_=den[:])
                outb = small.tile([P, BATCH, C], f32, tag="outb")
                nc.vector.tensor_tensor(
                    out=outb[:],
                    in0=num[:],
                    in1=invd[:].unsqueeze(-1).to_broadcast([P, BATCH, C]),
                    op=mybir.AluOpType.mult,
                )
                nc.sync.dma_start(out_v[:, b * BATCH:(b + 1) * BATCH, :], outb[:])

    nc.finalize()
    return nc


def _bf16(a):
    import ml_dtypes

    return a.astype(ml_dtypes.bfloat16).astype(np.float32)


def _split3(a):
    """fp32 -> (hi, mid, lo) bf16-representable fp32 triplet, a ~= hi+mid+lo."""
    h = _bf16(a)
    r = (a - h).astype(np.float32)
    m = _bf16(r)
    l = _bf16((r - m).astype(np.float32))
    return h, m, l


def _prep_inputs(x, pos_x, pos_y):
    """Build compensated-bf16 matmul operands.

    Score s = 2*y.x - ||x||^2 is computed on the PE as a K=21 bf16 matmul:
    products {yh*xh, yh*xm, ym*xh, ym*xm, yh*xl, yl*xh} per coordinate plus a
    3-way split of -||x||^2 against a ones row.  Rows are ordered small
    magnitude first so fp32 PSUM accumulation rounds on small partials; total
    score error ~3e-7, comparable to the fp32 reference's own rounding.
    """
    import ml_dtypes

    x = np.ascontiguousarray(x, dtype=np.float32)
    pos_x = np.ascontiguousarray(pos_x, dtype=np.float32)
    pos_y = np.ascontiguousarray(pos_y, dtype=np.float32)

    xsq = (pos_x * pos_x).sum(axis=-1, dtype=np.float32)  # [NX]
    xh, xm, xl = _split3(2.0 * pos_x.T)                   # each [3, NX]
    sh, sm, sl = _split3(-xsq[None, :])                   # each [1, NX]
    # row order (small->large): hl(3) lh(3) mm(3) sl(1) hm(3) mh(3) sm(1)
    #                           hh(3) sh(1)
    xt_rows = [xl, xh, xm, sl, xm, xh, sm, xh, sh]

    in_maps = []
    bfdt = ml_dtypes.bfloat16
    xt = np.ascontiguousarray(np.concatenate(xt_rows, axis=0)).astype(bfdt)
    for core in range(N_CORES):
        ys = pos_y[core * NY_SHARD:(core + 1) * NY_SHARD]  # [NY_SHARD, 3]
        yh, ym, yl = _split3(ys.T)                         # each [3, NY_SHARD]
        ones = np.ones((1, NY_SHARD), dtype=np.float32)
        yt_rows = [yh, yl, ym, ones, yh, ym, ones, yh, ones]
        yt = np.ascontiguousarray(np.concatenate(yt_rows, axis=0)).astype(bfdt)
        ysq = (ys * ys).sum(axis=-1, dtype=np.float32)  # [NY_SHARD]
        ysq_t = np.ascontiguousarray(ysq.reshape(N_CHUNKS, P).T)  # [P, N_CHUNKS]
        in_maps.append({"yt": yt, "xt": xt, "ysq": ysq_t, "xf": x})
    return in_maps


def _get_callable():
    """Build the PJRT executable once (mirrors bass2jax.run_bass_via_pjrt)."""
    global _BUILT
    if _BUILT is not None:
        return _BUILT

    import jax
    from jax.sharding import Mesh, PartitionSpec
    from jax.experimental.shard_map import shard_map
    from concourse import bass2jax
    from concourse import mybir as mb

    nc = _build_kernel()
    bass2jax.install_neuronx_cc_hook()

    partition_name = (
        nc.partition_id_tensor.name if nc.partition_id_tensor else None
    )
    in_names, out_names, out_avals, zero_outs = [], [], [], []
    for alloc in nc.m.functions[0].allocations:
        if not isinstance(alloc, mb.MemoryLocationSet):
            continue
        name = alloc.memorylocations[0].name
        if alloc.kind == "ExternalInput":
            if name != partition_name:
                in_names.append(name)
        elif alloc.kind == "ExternalOutput":
            shape = tuple(alloc.tensor_shape)
            dtype = mb.dt.np(alloc.dtype)
            out_names.append(name)
            out_avals.append(jax.core.ShapedArray(shape, dtype))
            zero_outs.append(np.zeros(shape, dtype))
    n_params = len(in_names)
    n_outs = len(out_avals)
    all_in_names = list(in_names) + list(out_names)
    if partition_name is not None:
        all_in_names.append(partition_name)
    donate = tuple(range(n_params, n_params + n_outs))

    def _body(*args):
        operands = list(args)
        if partition_name is not None:
            operands.append(bass2jax.partition_id_tensor())
        outs = bass2jax._bass_exec_p.bind(
            *operands,
            out_avals=tuple(out_avals),
            in_names=tuple(all_in_names),
            out_names=tuple(out_names),
            lowering_input_output_aliases=(),
            sim_require_finite=True,
            sim_require_nnan=True,
            nc=nc,
        )
        return tuple(outs)

    devices = jax.devices()[:N_CORES]
    mesh = Mesh(np.asarray(devices), ("core",))
    in_specs = (PartitionSpec("core"),) * (n_params + n_outs)
    out_specs = (PartitionSpec("core"),) * n_outs
    sharded = jax.jit(
        shard_map(
            _body, mesh=mesh, in_specs=in_specs, out_specs=out_specs,
            check_rep=False,
        ),
        donate_argnums=donate,
        keep_unused=True,
    )
    _BUILT = (sharded, in_names, out_names, zero_outs)
    return _BUILT


def _concat_inputs(in_maps, in_names):
    return [
        np.concatenate([m[name] for m in in_maps], axis=0) for name in in_names
    ]


def kernel(x, pos_x, pos_y, k):
    assert int(k) == K, f"kernel hardcodes k={K}, got {k}"
    sharded, in_names, out_names, zero_outs = _get_callable()

    in_maps = _prep_inputs(x, pos_x, pos_y)
    concat_in = _concat_inputs(in_maps, in_names)
    concat_zeros = [
        np.zeros((N_CORES * z.shape[0], *z.shape[1:]), z.dtype)
        for z in zero_outs
    ]
    out_arrs = sharded(*concat_in, *concat_zeros)
    out = np.asarray(out_arrs[out_names.index("out")])
    return out


def bench(x, pos_x, pos_y, iters=20):
    """Steady-state wall time of the device call with device-resident inputs."""
    import time
    import jax

    sharded, in_names, out_names, zero_outs = _get_callable()
    in_maps = _prep_inputs(x, pos_x, pos_y)
    concat_in = _concat_inputs(in_maps, in_names)
    dev_in = [jax.device_put(a) for a in concat_in]  # committed later by jit
    times = []
    for _ in range(iters):
        zeros = [
            np.zeros((N_CORES * z.shape[0], *z.shape[1:]), z.dtype)
            for z in zero_outs
        ]
        t0 = time.perf_counter()
        out = sharded(*dev_in, *zeros)
        jax.block_until_ready(out)
        times.append(time.perf_counter() - t0)
    return min(times), sum(times) / len(times)
